# revision 1
# baseline (speedup 1.0000x reference)
"""Trainium2 Bass kernel for nn_ChargesReadoutBlock.

Math: the reference is
    y_l = (x_l @ W_lin_l) / sqrt(256)            (per irrep l = 0e, 1o, 2e)
    p_l = einsum('zui,u,zi->z', y_l, W_tp_l, c_l) / sqrt(2l+1)
    energy = (p_0 + p_1 + p_2) / sqrt(3*256)

Everything is linear, so the two weight stages collapse on the host:
    w_l = W_lin_l @ W_tp_l                       ([256] each)
    energy[z] = sum_k node_feats[z, k] * A[k] * C[z, j(k)]
where A folds w_l and all scalar norms (per-k weight, shared across nodes)
and C[z, j] are the 9 per-node charge components (j = irrep slot of k).

Device kernel (data-parallel over nodes, 8 cores x 8192 nodes):
nodes live on partitions (128) x 64 node-tiles; for each node-tile the
DVE runs 9 fused scalar_tensor_tensor ops
    accum_out[z, j] = sum_u (x_group_j[z, u] * C[z, j]) * A_group_j[u]
(one per irrep component, each a 256-element strided dot) plus one
9-element reduce. All heavy traffic is the single streaming read of
node_feats (contiguous 73.7KB/partition DMA chunks).
"""

import numpy as np

N_NODES = 65536
N_CORES = 8
MUL = 256
K = 9 * MUL            # 2304 features per node
P = 128                # SBUF partitions
N_SHARD = N_NODES // N_CORES   # 8192 nodes per core
T = N_SHARD // P       # 64 node-tiles per partition (node = p*T + t)
G = 8                  # node-tiles per DMA chunk
SQ3 = float(np.sqrt(3.0))
SQ5 = float(np.sqrt(5.0))

_PROGRAM_CACHE = {}
LAST_RESULTS = None    # BassKernelResults of the most recent kernel() call


def build_program(t_tiles=T, g_chunk=G, repeat=1, skip_compute=False,
                  dma_once=False):
    """Build the SPMD Bass program (same program for every core).

    repeat > 1 unrolls the whole body `repeat` times (straight-line, same
    buffers, identical output) — used only by the timing harness to
    amortize host dispatch overhead out of the measurement.
    """
    import concourse.bass as bass
    import concourse.tile as tile
    from concourse import mybir

    f32 = mybir.dt.float32
    mult = mybir.AluOpType.mult

    nc = bass.Bass(trn_type="TRN2", debug=False, name="charges_readout")
    x = nc.dram_tensor("x", [P, t_tiles * K], f32, kind="ExternalInput").ap()
    arep = nc.dram_tensor("arep", [P, K], f32, kind="ExternalInput").ap()
    ch = nc.dram_tensor("ch", [P, t_tiles * 9], f32, kind="ExternalInput").ap()
    en = nc.dram_tensor("en", [P, t_tiles], f32, kind="ExternalOutput").ap()

    n_chunks = t_tiles // g_chunk

    with tile.TileContext(nc) as tc:
        with tc.tile_pool(name="const", bufs=1) as cpool, \
             tc.tile_pool(name="xp", bufs=2) as xpool, \
             tc.tile_pool(name="scrp", bufs=3) as spool, \
             tc.tile_pool(name="s9p", bufs=4) as s9pool:

            arep_t = cpool.tile([P, K], f32)
            nc.sync.dma_start(out=arep_t[:], in_=arep[:, :])
            ch_t = cpool.tile([P, t_tiles * 9], f32)
            nc.sync.dma_start(out=ch_t[:], in_=ch[:, :])
            en_t = cpool.tile([P, t_tiles], f32)

            # Wait-collectors: absorb the const-DMA completion waits on cheap
            # copy ops so the first scalar_tensor_tensor doesn't accumulate
            # more sync-wait slots than its ISA struct allows.
            dummy = cpool.tile([P, 2], f32)
            nc.vector.tensor_copy(dummy[:, 0:1], arep_t[:, 0:1])
            nc.vector.tensor_copy(dummy[:, 1:2], ch_t[:, 0:1])
            if skip_compute:
                nc.vector.memset(en_t[:], 0.0)

            # Strided per-component views of the folded weights: for l=1 the
            # section layout is k = 3u+i, so component i is a stride-3 view.
            a0 = arep_t[:, 0:MUL]
            a1 = arep_t[:, MUL:4 * MUL].rearrange("p (u i) -> p i u", i=3)
            a2 = arep_t[:, 4 * MUL:9 * MUL].rearrange("p (u i) -> p i u", i=5)

            xg_cache = []
            for _rep in range(repeat):
                for c in range(n_chunks):
                    # dma_once: load only the first two chunks, then cycle
                    # those buffers for all compute (DVE-only timing probe).
                    if dma_once and len(xg_cache) >= 2:
                        xg = xg_cache[c % 2]
                    else:
                        xg = xpool.tile([P, g_chunk * K], f32)
                        nc.sync.dma_start(
                            out=xg[:],
                            in_=x[:, c * g_chunk * K:(c + 1) * g_chunk * K]
                        )
                        if dma_once:
                            xg_cache.append(xg)
                    if skip_compute:
                        # tiny DVE read per chunk keeps the reader-release
                        # wait chain intact (prunable single-wait DMAs)
                        sink = spool.tile([P, 1], f32)
                        nc.vector.tensor_copy(sink[:], xg[:, 0:1])
                        continue
                    s9c = s9pool.tile([P, g_chunk * 9], f32)
                    for tsub in range(g_chunk):
                        t_idx = c * g_chunk + tsub
                        base = tsub * K
                        # per-group scratch slices: no WAW between the 9
                        # STTs of a tile, so no self-waits on DVE
                        scr9 = spool.tile([P, 9 * MUL], f32)
                        x0 = xg[:, base:base + MUL]
                        x1 = xg[:, base + MUL:base + 4 * MUL].rearrange(
                            "p (u i) -> p i u", i=3)
                        x2 = xg[:, base + 4 * MUL:base + 9 * MUL].rearrange(
                            "p (u i) -> p i u", i=5)
                        groups = [(x0, a0, 0)]
                        groups += [(x1[:, i, :], a1[:, i, :], 1 + i)
                                   for i in range(3)]
                        groups += [(x2[:, i, :], a2[:, i, :], 4 + i)
                                   for i in range(5)]
                        for xi, ai, j in groups:
                            nc.vector.scalar_tensor_tensor(
                                out=scr9[:, j * MUL:(j + 1) * MUL],
                                in0=xi,
                                scalar=ch_t[:, t_idx * 9 + j:t_idx * 9 + j + 1],
                                in1=ai,
                                op0=mult,
                                op1=mult,
                                accum_out=s9c[:, tsub * 9 + j:tsub * 9 + j + 1],
                            )
                    # one batched reduce per chunk: [P, g, 9] -> [P, g]
                    nc.vector.tensor_reduce(
                        out=en_t[:, c * g_chunk:(c + 1) * g_chunk],
                        in_=s9c[:].rearrange("p (g j) -> p g j", j=9),
                        axis=mybir.AxisListType.X,
                        op=mybir.AluOpType.add,
                    )
            nc.sync.dma_start(out=en[:, :], in_=en_t[:])
    _prune_implied_dma_waits(nc)
    return nc


def _prune_implied_dma_waits(nc):
    """Drop transitively-implied DMA-lane waits from DMACopy instructions.

    Walrus in this toolchain rejects DMAs with more than one sync wait.
    Tile emits (a) the reader-release wait on the DVE proc sem and (b) WAW /
    lane-FIFO waits on DMA completion sems. (b) is redundant whenever an
    earlier DVE instruction already waited on the same (sem >= value) and
    that instruction completed within the DVE wait of (a) — the vector
    clocks make the DMA completion transitively ordered. Tile's wait
    emission is per-proc minimal but not transitively minimal (documented),
    so we do the reduction here, dropping only waits we can prove implied.
    """
    from concourse import mybir

    blocks = nc.m.functions[0].blocks
    # DVE instruction stream in program order with cumulative DVE-sem ticks
    # and the waits each instruction carried.
    dve_sem = None
    dve_stream = []   # (cum_ticks_after, [(sem_name, wait_value), ...])
    cum = 0
    for blk in blocks:
        for inst in blk.instructions:
            if inst.engine != mybir.EngineType.DVE:
                continue
            si = inst.sync_info
            waits = [(w.ant_name, w.wait_value) for w in (si.on_wait or [])] \
                if si else []
            if si and si.on_update:
                for u in si.on_update:
                    if u.ant_name.startswith("DVE"):
                        dve_sem = u.ant_name
                        cum += u.update_value
            dve_stream.append((cum, waits))

    # Per DMA-lane sem: (cumulative completion value, waits the DMA carried),
    # in program order.
    dma_lane = {}
    for blk in blocks:
        for inst in blk.instructions:
            if inst.opcode != "DMACopy":
                continue
            si = inst.sync_info
            if not si or not si.on_update:
                continue
            waits = [(w.ant_name, w.wait_value) for w in (si.on_wait or [])]
            for u in si.on_update:
                lane = dma_lane.setdefault(u.ant_name, [])
                prev = lane[-1][0] if lane else 0
                lane.append((prev + u.update_value, waits))

    def implied_by_dve(sem_name, value, dve_target):
        # DVE sem >= dve_target => the DVE instruction pushing it there (and
        # all earlier DVE instructions, engine is in-order) completed, so
        # every wait they carried is satisfied.
        for cum_after, waits in dve_stream:
            for s, v in waits:
                if s == sem_name and v >= value:
                    return True
            if cum_after >= dve_target:
                break
        return False

    def implied_by(w, other, depth=0):
        """Is wait w (sem >= value) implied by `other` being satisfied?"""
        s, v = w
        os, ov = other
        if os == dve_sem:
            return implied_by_dve(s, v, ov)
        if os in dma_lane:
            # other satisfied => all DMAs on that lane up to value ov
            # completed => their own waits were satisfied beforehand.
            for cum, waits in dma_lane[os]:
                for ww in waits:
                    if ww[0] == s and ww[1] >= v:
                        return True
                    if depth < 2 and implied_by(w, ww, depth + 1):
                        return True
                if cum >= ov:
                    break
        return False

    for blk in blocks:
        for inst in blk.instructions:
            if inst.opcode not in ("DMACopy", "Drain"):
                continue
            si = inst.sync_info
            if not si or not si.on_wait or len(si.on_wait) <= 1:
                continue
            waits = [(w.ant_name, w.wait_value) for w in si.on_wait]
            kept_idx = list(range(len(waits)))
            changed = True
            while changed:
                changed = False
                for i in list(kept_idx):
                    others = [waits[j] for j in kept_idx if j != i]
                    if any(implied_by(waits[i], o) for o in others):
                        kept_idx.remove(i)
                        changed = True
                        break
            si.on_wait = [si.on_wait[i] for i in kept_idx]
    return nc


def _get_program():
    key = (T, G)
    if key not in _PROGRAM_CACHE:
        _PROGRAM_CACHE[key] = build_program()
    return _PROGRAM_CACHE[key]


def fold_weights(W_lin0, W_lin1, W_lin2, W_tp0, W_tp1, W_tp2):
    """Collapse both weight stages + norms into one per-k weight A[2304]."""
    lin_norm = 1.0 / np.sqrt(np.float64(MUL))
    alpha = 1.0 / np.sqrt(3.0 * MUL)
    w0 = W_lin0.astype(np.float64) @ W_tp0.astype(np.float64)
    w1 = W_lin1.astype(np.float64) @ W_tp1.astype(np.float64)
    w2 = W_lin2.astype(np.float64) @ W_tp2.astype(np.float64)
    A = np.empty(K, np.float64)
    A[0:MUL] = w0 * (alpha * lin_norm)
    A[MUL:4 * MUL] = np.repeat(w1 * (alpha * lin_norm / SQ3), 3)
    A[4 * MUL:9 * MUL] = np.repeat(w2 * (alpha * lin_norm / SQ5), 5)
    return A.astype(np.float32)


def kernel(**inputs):
    global LAST_RESULTS
    from concourse.bass_utils import run_bass_kernel_spmd

    node_feats = np.asarray(inputs["node_feats"], dtype=np.float32)
    charges = np.asarray(inputs["charges"], dtype=np.float32)
    A = fold_weights(
        np.asarray(inputs["W_lin0"], dtype=np.float32),
        np.asarray(inputs["W_lin1"], dtype=np.float32),
        np.asarray(inputs["W_lin2"], dtype=np.float32),
        np.asarray(inputs["W_tp0"], dtype=np.float32),
        np.asarray(inputs["W_tp1"], dtype=np.float32),
        np.asarray(inputs["W_tp2"], dtype=np.float32),
    )
    arep = np.ascontiguousarray(np.broadcast_to(A, (P, K)))

    node_feats = np.ascontiguousarray(node_feats)
    charges = np.ascontiguousarray(charges)

    in_maps = []
    for c in range(N_CORES):
        lo, hi = c * N_SHARD, (c + 1) * N_SHARD
        in_maps.append({
            "x": node_feats[lo:hi].reshape(P, T * K),
            "arep": arep,
            "ch": charges[lo:hi].reshape(P, T * 9),
        })

    nc = _get_program()
    res = run_bass_kernel_spmd(nc, in_maps, list(range(N_CORES)))
    LAST_RESULTS = res
    out = np.concatenate(
        [np.asarray(res.results[c]["en"]).reshape(N_SHARD) for c in range(N_CORES)]
    )
    return out



# revision 12
# speedup vs baseline: 1.5766x; 1.5766x over previous
"""Trainium2 Bass kernel for nn_ChargesReadoutBlock.

Math: the reference is
    y_l = (x_l @ W_lin_l) / sqrt(256)            (per irrep l = 0e, 1o, 2e)
    p_l = einsum('zui,u,zi->z', y_l, W_tp_l, c_l) / sqrt(2l+1)
    energy = (p_0 + p_1 + p_2) / sqrt(3*256)

Everything is linear, so the two weight stages collapse on the host:
    w_l = W_lin_l @ W_tp_l                       ([256] each)
    energy[z] = sum_k node_feats[z, k] * A[k] * C[z, j(k)]
where A folds w_l and all scalar norms (per-k weight, shared across nodes)
and C[z, j] are the 9 per-node charge components (j = irrep slot of k).

Device kernel (data-parallel over nodes, 8 cores x 8192 nodes):
nodes live on partitions (128) x 64 node-tiles. Host-side repack (pure
permutation, no FLOPs):
  * columns de-interleaved to grouped layout [j][u] so every DVE operand
    is stride-1 (the e3nn layout stores l=1/2 components interleaved,
    k = 3u+i, which would force stride-3/5 reads);
  * each g_chunk-tile DMA chunk is stored contiguously in DRAM
    ([n_chunks, P, g*K] layout) so a chunk DMA is one dense 9.4 MB
    sequential HBM read instead of 128 strided 590KB-apart streams.
For each node-tile the DVE runs 9 fused scalar_tensor_tensor ops
    accum_out[z, j] = sum_u (x_group_j[z, u] * C[z, j]) * A_group_j[u]
plus one batched 9-element reduce per chunk. Chunk DMAs alternate
between the two HWDGE queues (sync / scalar engines).
"""

import numpy as np

N_NODES = 65536
N_CORES = 8
MUL = 256
K = 9 * MUL            # 2304 features per node
P = 128                # SBUF partitions
N_SHARD = N_NODES // N_CORES   # 8192 nodes per core
T = N_SHARD // P       # 64 node-tiles per partition (node = p*T + t)
G = 8                  # node-tiles per DMA chunk
SQ3 = float(np.sqrt(3.0))
SQ5 = float(np.sqrt(5.0))

_PROGRAM_CACHE = {}
LAST_RESULTS = None    # BassKernelResults of the most recent kernel() call


def build_program_bf16(t_tiles=T, g_chunk=G, repeat=1, skip_compute=False,
                       dma_once=False, act_j=(4, 5, 6, 7, 8)):
    """bf16 pipeline: SWDGE cast-DMA (fp32 HBM -> bf16 SBUF), one
    tensor_tensor multiply by the replicated A-pattern per chunk (2x mode),
    then per-(tile, irrep-slot) tensor_scalar ops with scalar=C and
    accum_out (4x mode) for the segmented dots. Slots in act_j run on the
    scalar engine (activation Copy, scale=C, accum_out) instead of the DVE
    so the two engines split the 72 per-chunk group-reductions.

    x DRAM layout matches build_program: [n_chunks * P, g*K] fp32 dense
    chunks, grouped columns. arep8 is the A-pattern replicated g times
    (bf16). Output en is fp32 [P, t_tiles].
    """
    import concourse.bass as bass
    import concourse.tile as tile
    from concourse import mybir

    f32 = mybir.dt.float32
    bf16 = mybir.dt.bfloat16
    mult = mybir.AluOpType.mult
    copyf = mybir.ActivationFunctionType.Copy

    n_chunks = t_tiles // g_chunk

    nc = bass.Bass(trn_type="TRN2", debug=False, name="charges_readout")
    x = nc.dram_tensor("x", [n_chunks * P, g_chunk * K], f32,
                       kind="ExternalInput").ap()
    arep8 = nc.dram_tensor("arep8", [P, g_chunk * K], bf16,
                           kind="ExternalInput").ap()
    ch = nc.dram_tensor("ch", [P, t_tiles * 9], f32, kind="ExternalInput").ap()
    en = nc.dram_tensor("en", [P, t_tiles], f32, kind="ExternalOutput").ap()

    with tile.TileContext(nc) as tc:
        n_act = len(act_j)
        n_dve = 9 - n_act
        with tc.tile_pool(name="const", bufs=1) as cpool, \
             tc.tile_pool(name="xp", bufs=2) as xpool, \
             tc.tile_pool(name="xap", bufs=2) as xapool, \
             tc.tile_pool(name="scrd", bufs=2) as sdpool, \
             tc.tile_pool(name="scra", bufs=2) as sapool, \
             tc.tile_pool(name="s9p", bufs=4) as s9pool:

            arep_t = cpool.tile([P, g_chunk * K], bf16)
            nc.sync.dma_start(out=arep_t[:], in_=arep8[:, :])
            ch_t = cpool.tile([P, t_tiles * 9], f32)
            nc.sync.dma_start(out=ch_t[:], in_=ch[:, :])
            en_t = cpool.tile([P, t_tiles], f32)

            # Wait-collectors: absorb const-DMA completion waits on cheap
            # copies so compute ops carry at most one sync wait each.
            dummy = cpool.tile([P, 2], f32)
            nc.vector.tensor_copy(dummy[:, 0:1], arep_t[:, 0:1])
            nc.vector.tensor_copy(dummy[:, 1:2], ch_t[:, 0:1])
            if skip_compute:
                nc.vector.memset(en_t[:], 0.0)
            else:
                dummy_a = cpool.tile([P, 1], f32)
                nc.scalar.activation(dummy_a[:], ch_t[:, 0:1], copyf)

            xg_cache = []
            for _rep in range(repeat):
                for c in range(n_chunks):
                    if dma_once and len(xg_cache) >= 2:
                        xg = xg_cache[c % 2]
                    else:
                        xg = xpool.tile([P, g_chunk * K], bf16)
                        nc.gpsimd.dma_start(
                            out=xg[:],
                            in_=x[c * P:(c + 1) * P, :],
                        )
                        if dma_once:
                            xg_cache.append(xg)
                    if skip_compute:
                        nc.vector.tensor_copy(en_t[:, c:c + 1], xg[:, 0:1])
                        continue
                    # xa = xg * A  (one 2x-mode TT over the whole chunk)
                    xa = xapool.tile([P, g_chunk * K], bf16)
                    nc.vector.tensor_tensor(
                        out=xa[:], in0=xg[:], in1=arep_t[:], op=mult)
                    s9c = s9pool.tile([P, g_chunk * 9], f32)
                    for tsub in range(g_chunk):
                        t_idx = c * g_chunk + tsub
                        base = tsub * K
                        scrd = sdpool.tile([P, max(n_dve, 1) * MUL], bf16)
                        scra = sapool.tile([P, max(n_act, 1) * MUL], bf16)
                        id_, ia = 0, 0
                        for j in range(9):
                            xa_g = xa[:, base + j * MUL:base + (j + 1) * MUL]
                            ch_s = ch_t[:, t_idx * 9 + j:t_idx * 9 + j + 1]
                            acc = s9c[:, tsub * 9 + j:tsub * 9 + j + 1]
                            if j in act_j:
                                nc.scalar.activation(
                                    out=scra[:, ia * MUL:(ia + 1) * MUL],
                                    in_=xa_g, func=copyf, scale=ch_s,
                                    accum_out=acc)
                                ia += 1
                            else:
                                nc.vector.tensor_scalar(
                                    out=scrd[:, id_ * MUL:(id_ + 1) * MUL],
                                    in0=xa_g, scalar1=ch_s, scalar2=None,
                                    op0=mult, accum_out=acc)
                                id_ += 1
                    nc.vector.tensor_reduce(
                        out=en_t[:, c * g_chunk:(c + 1) * g_chunk],
                        in_=s9c[:].rearrange("p (g j) -> p g j", j=9),
                        axis=mybir.AxisListType.X,
                        op=mybir.AluOpType.add,
                    )
            nc.sync.dma_start(out=en[:, :], in_=en_t[:])
    _prune_implied_dma_waits(nc)
    return nc


def build_program(t_tiles=T, g_chunk=G, repeat=1, skip_compute=False,
                  dma_once=False, two_queues=True):
    """Build the SPMD Bass program (same program for every core).

    x DRAM layout: [n_chunks * P, g*K] — chunk c occupies rows
    [c*P, (c+1)*P), so each chunk DMA reads a dense contiguous region.

    repeat > 1 unrolls the whole body `repeat` times (straight-line, same
    buffers, identical output) — used only by the timing harness to
    amortize host dispatch overhead out of the measurement.
    """
    import concourse.bass as bass
    import concourse.tile as tile
    from concourse import mybir

    f32 = mybir.dt.float32
    mult = mybir.AluOpType.mult

    n_chunks = t_tiles // g_chunk

    nc = bass.Bass(trn_type="TRN2", debug=False, name="charges_readout")
    x = nc.dram_tensor("x", [n_chunks * P, g_chunk * K], f32,
                       kind="ExternalInput").ap()
    arep = nc.dram_tensor("arep", [P, K], f32, kind="ExternalInput").ap()
    ch = nc.dram_tensor("ch", [P, t_tiles * 9], f32, kind="ExternalInput").ap()
    en = nc.dram_tensor("en", [P, t_tiles], f32, kind="ExternalOutput").ap()

    with tile.TileContext(nc) as tc:
        with tc.tile_pool(name="const", bufs=1) as cpool, \
             tc.tile_pool(name="xp", bufs=2) as xpool, \
             tc.tile_pool(name="scrp", bufs=3) as spool, \
             tc.tile_pool(name="s9p", bufs=4) as s9pool:

            arep_t = cpool.tile([P, K], f32)
            nc.sync.dma_start(out=arep_t[:], in_=arep[:, :])
            ch_t = cpool.tile([P, t_tiles * 9], f32)
            nc.sync.dma_start(out=ch_t[:], in_=ch[:, :])
            en_t = cpool.tile([P, t_tiles], f32)

            # Wait-collectors: absorb the const-DMA completion waits on cheap
            # copy ops so the first scalar_tensor_tensor doesn't accumulate
            # more sync-wait slots than its ISA struct allows.
            dummy = cpool.tile([P, 2], f32)
            nc.vector.tensor_copy(dummy[:, 0:1], arep_t[:, 0:1])
            nc.vector.tensor_copy(dummy[:, 1:2], ch_t[:, 0:1])
            if skip_compute:
                nc.vector.memset(en_t[:], 0.0)

            # Grouped per-component views of the folded weights: component j
            # occupies the contiguous 256-column block j.
            a_g = [arep_t[:, j * MUL:(j + 1) * MUL] for j in range(9)]

            xg_cache = []
            for _rep in range(repeat):
                for c in range(n_chunks):
                    dma_eng = nc.scalar if (two_queues and c % 2) else nc.sync
                    # dma_once: load only the first two chunks, then cycle
                    # those buffers for all compute (DVE-only timing probe).
                    if dma_once and len(xg_cache) >= 2:
                        xg = xg_cache[c % 2]
                    else:
                        xg = xpool.tile([P, g_chunk * K], f32)
                        dma_eng.dma_start(
                            out=xg[:],
                            in_=x[c * P:(c + 1) * P, :],
                        )
                        if dma_once:
                            xg_cache.append(xg)
                    if skip_compute:
                        # tiny DVE read per chunk keeps the reader-release
                        # wait chain intact; writing into en_t makes the
                        # final out-DMA (and transitively the Drain) cover
                        # the sink chain with a single wait.
                        nc.vector.tensor_copy(en_t[:, c:c + 1], xg[:, 0:1])
                        continue
                    s9c = s9pool.tile([P, g_chunk * 9], f32)
                    for tsub in range(g_chunk):
                        t_idx = c * g_chunk + tsub
                        base = tsub * K
                        # per-group scratch slices: no WAW between the 9
                        # STTs of a tile, so no self-waits on DVE
                        scr9 = spool.tile([P, 9 * MUL], f32)
                        for j in range(9):
                            nc.vector.scalar_tensor_tensor(
                                out=scr9[:, j * MUL:(j + 1) * MUL],
                                in0=xg[:, base + j * MUL:base + (j + 1) * MUL],
                                scalar=ch_t[:, t_idx * 9 + j:t_idx * 9 + j + 1],
                                in1=a_g[j],
                                op0=mult,
                                op1=mult,
                                accum_out=s9c[:, tsub * 9 + j:tsub * 9 + j + 1],
                            )
                    # one batched reduce per chunk: [P, g, 9] -> [P, g]
                    nc.vector.tensor_reduce(
                        out=en_t[:, c * g_chunk:(c + 1) * g_chunk],
                        in_=s9c[:].rearrange("p (g j) -> p g j", j=9),
                        axis=mybir.AxisListType.X,
                        op=mybir.AluOpType.add,
                    )
            nc.sync.dma_start(out=en[:, :], in_=en_t[:])
    _prune_implied_dma_waits(nc)
    return nc


def _prune_implied_dma_waits(nc):
    """Drop transitively-implied DMA-lane waits from DMACopy instructions.

    Walrus in this toolchain rejects DMAs with more than one sync wait.
    Tile emits (a) the reader-release wait on the DVE proc sem and (b) WAW /
    lane-FIFO waits on DMA completion sems. (b) is redundant whenever an
    earlier DVE instruction already waited on the same (sem >= value) and
    that instruction completed within the DVE wait of (a) — the vector
    clocks make the DMA completion transitively ordered. Tile's wait
    emission is per-proc minimal but not transitively minimal (documented),
    so we do the reduction here, dropping only waits we can prove implied.
    """
    from concourse import mybir

    blocks = nc.m.functions[0].blocks
    # DVE instruction stream in program order with cumulative DVE-sem ticks
    # and the waits each instruction carried.
    dve_sem = None
    dve_stream = []   # (cum_ticks_after, [(sem_name, wait_value), ...])
    cum = 0
    for blk in blocks:
        for inst in blk.instructions:
            if inst.engine != mybir.EngineType.DVE:
                continue
            si = inst.sync_info
            waits = [(w.ant_name, w.wait_value) for w in (si.on_wait or [])] \
                if si else []
            if si and si.on_update:
                for u in si.on_update:
                    if u.ant_name.startswith("DVE"):
                        dve_sem = u.ant_name
                        cum += u.update_value
            dve_stream.append((cum, waits))

    # Per DMA-lane sem: (cumulative completion value, waits the DMA carried),
    # in program order.
    dma_lane = {}
    for blk in blocks:
        for inst in blk.instructions:
            if inst.opcode != "DMACopy":
                continue
            si = inst.sync_info
            if not si or not si.on_update:
                continue
            waits = [(w.ant_name, w.wait_value) for w in (si.on_wait or [])]
            for u in si.on_update:
                lane = dma_lane.setdefault(u.ant_name, [])
                prev = lane[-1][0] if lane else 0
                lane.append((prev + u.update_value, waits))

    def implied_by_dve(sem_name, value, dve_target):
        # DVE sem >= dve_target => the DVE instruction pushing it there (and
        # all earlier DVE instructions, engine is in-order) completed, so
        # every wait they carried is satisfied.
        for cum_after, waits in dve_stream:
            for s, v in waits:
                if s == sem_name and v >= value:
                    return True
            if cum_after >= dve_target:
                break
        return False

    def implied_by(w, other, depth=0):
        """Is wait w (sem >= value) implied by `other` being satisfied?"""
        s, v = w
        os, ov = other
        if os == dve_sem:
            return implied_by_dve(s, v, ov)
        if os in dma_lane:
            # other satisfied => all DMAs on that lane up to value ov
            # completed => their own waits were satisfied beforehand.
            for cum, waits in dma_lane[os]:
                for ww in waits:
                    if ww[0] == s and ww[1] >= v:
                        return True
                    if depth < 2 and implied_by(w, ww, depth + 1):
                        return True
                if cum >= ov:
                    break
        return False

    # Engines sometimes carry a wait on their own proc sem with a value
    # already reached by their preceding in-order instructions — trivially
    # satisfied (the sem can only be at/above the engine-local count), and
    # it costs a precious wait slot. Drop those, for every engine.
    cum_by = {}   # (engine, sem_name) -> cumulative synchronous updates
    for blk in blocks:
        for inst in blk.instructions:
            eng = inst.engine
            si = inst.sync_info
            if si and si.on_wait:
                kept = [w for w in si.on_wait
                        if cum_by.get((eng, w.ant_name), -1) < w.wait_value]
                if len(kept) != len(si.on_wait):
                    si.on_wait = kept
            # DMACopy sem updates fire at DMA *completion*, asynchronously —
            # they say nothing about engine-local program order. Skip them.
            if si and si.on_update and inst.opcode != "DMACopy":
                for u in si.on_update:
                    key = (eng, u.ant_name)
                    cum_by[key] = cum_by.get(key, 0) + u.update_value

    for blk in blocks:
        for inst in blk.instructions:
            if inst.opcode not in ("DMACopy", "Drain"):
                continue
            si = inst.sync_info
            if not si or not si.on_wait or len(si.on_wait) <= 1:
                continue
            waits = [(w.ant_name, w.wait_value) for w in si.on_wait]
            kept_idx = list(range(len(waits)))
            changed = True
            while changed:
                changed = False
                for i in list(kept_idx):
                    others = [waits[j] for j in kept_idx if j != i]
                    if any(implied_by(waits[i], o) for o in others):
                        kept_idx.remove(i)
                        changed = True
                        break
            si.on_wait = [si.on_wait[i] for i in kept_idx]
    return nc


PIPELINE = "bf16"          # "bf16" or "fp32"


def _get_program():
    key = (T, G, PIPELINE)
    if key not in _PROGRAM_CACHE:
        builder = build_program_bf16 if PIPELINE == "bf16" else build_program
        _PROGRAM_CACHE[key] = builder()
    return _PROGRAM_CACHE[key]


def fold_weights(W_lin0, W_lin1, W_lin2, W_tp0, W_tp1, W_tp2):
    """Collapse both weight stages + norms into one per-k weight A[2304],
    in GROUPED column order (component-major: [j][u], j = irrep slot)."""
    lin_norm = 1.0 / np.sqrt(np.float64(MUL))
    alpha = 1.0 / np.sqrt(3.0 * MUL)
    w0 = W_lin0.astype(np.float64) @ W_tp0.astype(np.float64)
    w1 = W_lin1.astype(np.float64) @ W_tp1.astype(np.float64)
    w2 = W_lin2.astype(np.float64) @ W_tp2.astype(np.float64)
    A = np.empty(K, np.float64)
    A[0:MUL] = w0 * (alpha * lin_norm)
    for i in range(3):
        A[(1 + i) * MUL:(2 + i) * MUL] = w1 * (alpha * lin_norm / SQ3)
    for i in range(5):
        A[(4 + i) * MUL:(5 + i) * MUL] = w2 * (alpha * lin_norm / SQ5)
    return A.astype(np.float32)


def regroup_columns(x):
    """De-interleave e3nn columns to grouped layout: out[:, j*256+u] =
    x[:, k(j,u)] with k = u | 256+3u+i | 1024+5u+i. Pure permutation."""
    n = x.shape[0]
    out = np.empty_like(x)
    out[:, :MUL] = x[:, :MUL]
    out[:, MUL:4 * MUL] = (
        x[:, MUL:4 * MUL].reshape(n, MUL, 3).transpose(0, 2, 1)
        .reshape(n, 3 * MUL))
    out[:, 4 * MUL:] = (
        x[:, 4 * MUL:].reshape(n, MUL, 5).transpose(0, 2, 1)
        .reshape(n, 5 * MUL))
    return out


def make_in_maps(inputs):
    """Shard + repack full inputs into per-core in_maps for the program."""
    node_feats = np.ascontiguousarray(
        np.asarray(inputs["node_feats"], dtype=np.float32))
    charges = np.ascontiguousarray(
        np.asarray(inputs["charges"], dtype=np.float32))
    A = fold_weights(
        np.asarray(inputs["W_lin0"], dtype=np.float32),
        np.asarray(inputs["W_lin1"], dtype=np.float32),
        np.asarray(inputs["W_lin2"], dtype=np.float32),
        np.asarray(inputs["W_tp0"], dtype=np.float32),
        np.asarray(inputs["W_tp1"], dtype=np.float32),
        np.asarray(inputs["W_tp2"], dtype=np.float32),
    )
    arep = np.ascontiguousarray(np.broadcast_to(A, (P, K)))

    if PIPELINE == "bf16":
        from concourse import mybir
        bf = mybir.dt.np(mybir.dt.bfloat16)
        a8 = np.tile(A, G).astype(bf)              # A-pattern, replicated G x
        arep_w = {"arep8": np.ascontiguousarray(
            np.broadcast_to(a8, (P, G * K)))}
    else:
        arep_w = {"arep": arep}

    xg_all = regroup_columns(node_feats)      # [N, K] grouped columns
    n_chunks = T // G
    in_maps = []
    for c in range(N_CORES):
        lo, hi = c * N_SHARD, (c + 1) * N_SHARD
        shard = xg_all[lo:hi]                 # [8192, K], node = p*T + t
        xr = np.ascontiguousarray(
            shard.reshape(P, n_chunks, G * K).transpose(1, 0, 2)
        ).reshape(n_chunks * P, G * K)
        in_maps.append({
            "x": xr,
            "ch": charges[lo:hi].reshape(P, T * 9),
            **arep_w,
        })
    return in_maps


def kernel(**inputs):
    global LAST_RESULTS
    from concourse.bass_utils import run_bass_kernel_spmd

    in_maps = make_in_maps(inputs)
    nc = _get_program()
    res = run_bass_kernel_spmd(nc, in_maps, list(range(N_CORES)))
    LAST_RESULTS = res
    out = np.concatenate(
        [np.asarray(res.results[c]["en"]).reshape(N_SHARD) for c in range(N_CORES)]
    )
    return out
